# revision 5
# baseline (speedup 1.0000x reference)
"""3x3 valid conv (cross-correlation) + bias on a 4096x4096 f32 image.

Strategy: shard rows across 8 NeuronCores (512 output rows each, with a
2-row halo overlap provided host-side). On each core the conv is computed
on the tensor engine: for each column shift b in {0,1,2}, a banded
[K=M+2, M] matrix B_b with B_b[m+a, m] = w[a, b] folds all three row taps
into the K-contraction, so

    (B_b.T @ X_rows)[m, n] = sum_a w[a, b] * X[m+a, n]

and accumulating the three column shifts of the moving tensor into one
PSUM bank yields the full 3x3 conv. fp32r matmuls run at 1 cycle/row
(vs 4 for fp32) with ~1e-5..1e-4 relative error. Bias is fused into the
PSUM->SBUF eviction on the scalar engine.
"""

import sys

if "/opt/trn_rl_repo" not in sys.path:
    sys.path.insert(0, "/opt/trn_rl_repo")

import numpy as np

import concourse.bacc as bacc
import concourse.mybir as mybir
from concourse import tile
from concourse.bass_utils import run_bass_kernel_spmd

N_CORES = 8
H, W = 4096, 4096
KH, KW = 3, 3
HALO = 2  # KH - 1
OUT_ROWS = 512  # output rows per core (padded output H = 4096)
IN_ROWS = OUT_ROWS + HALO  # 514
W_PAD = W + HALO  # 4098: lets every core compute a full 4096-wide output
M_TILE = 126  # output rows per matmul (K = M + 2 <= 128)
N_TILE = 512  # matmul free dim = one PSUM bank of f32

_CACHE = {}


def _build_program():
    f32 = mybir.dt.float32
    f32r = mybir.dt.float32r

    nc = bacc.Bacc(
        "TRN2", target_bir_lowering=False, debug=False, num_devices=N_CORES
    )
    x = nc.declare_dram_parameter("x", [IN_ROWS, W_PAD], f32, isOutput=False)
    wb = nc.declare_dram_parameter("wb", [128, KW, M_TILE], f32, isOutput=False)
    bias = nc.declare_dram_parameter("bias", [128, 1], f32, isOutput=False)
    out = nc.declare_dram_parameter("out", [OUT_ROWS, W], f32, isOutput=True)

    # row groups: 4 x 126 + 1 x 8 = 512
    groups = []
    m0 = 0
    while m0 < OUT_ROWS:
        m = min(M_TILE, OUT_ROWS - m0)
        groups.append((m0, m))
        m0 += m

    n_cols = W // N_TILE  # 8 column tiles
    half = n_cols // 2
    xhalf = half * N_TILE + HALO  # 2050: input cols per half (2-col overlap)

    with tile.TileContext(nc) as tc:
        with (
            tc.tile_pool(name="const", bufs=1) as cpool,
            tc.tile_pool(name="xin", bufs=6) as xpool,
            tc.tile_pool(name="psum", bufs=8, space="PSUM") as ppool,
            tc.tile_pool(name="oput", bufs=6) as opool,
        ):
            wt = cpool.tile([128, KW, M_TILE], f32r)
            nc.sync.dma_start(wt[:], wb[:].bitcast(f32r))
            bt = cpool.tile([128, 1], f32)
            nc.sync.dma_start(bt[:], bias[:])

            for m0, m in groups:
                k = m + HALO
                for h in range(2):
                    # independent single-writer tiles per column half
                    xc0 = h * half * N_TILE  # 0 or 2048
                    xt = xpool.tile([128, xhalf], f32r, tag="xin")
                    nc.sync.dma_start(
                        xt[:k, :], x[m0 : m0 + k, xc0 : xc0 + xhalf].bitcast(f32r)
                    )
                    ot = opool.tile([128, half * N_TILE], f32, tag="oput")
                    for jj in range(half):
                        c0 = jj * N_TILE
                        pt = ppool.tile([128, N_TILE], f32)
                        for b in range(KW):
                            nc.tensor.matmul(
                                pt[:m, :],
                                wt[:k, b, :m],
                                xt[:k, c0 + b : c0 + b + N_TILE],
                                start=(b == 0),
                                stop=(b == KW - 1),
                            )
                        # evict psum+bias to SBUF; alternate ACT/DVE
                        if jj % 2 == 0:
                            nc.scalar.activation(
                                ot[:m, c0 : c0 + N_TILE],
                                pt[:m, :],
                                mybir.ActivationFunctionType.Identity,
                                bias=bt[:m],
                                scale=1.0,
                            )
                        else:
                            nc.vector.tensor_scalar_add(
                                ot[:m, c0 : c0 + N_TILE], pt[:m, :], bt[:m]
                            )
                    # stores go on the ACT HWDGE queue so their sem waits
                    # don't head-of-line-block input loads on the SP queue
                    nc.scalar.dma_start(
                        out[m0 : m0 + m, xc0 : xc0 + half * N_TILE], ot[:m, :]
                    )

    nc.compile()
    return nc


def kernel(X: np.ndarray, weight: np.ndarray, bias: np.ndarray) -> np.ndarray:
    X = np.ascontiguousarray(X, dtype=np.float32)
    weight = np.asarray(weight, dtype=np.float32)
    bias = np.asarray(bias, dtype=np.float32)

    if "nc" not in _CACHE:
        _CACHE["nc"] = _build_program()
    nc = _CACHE["nc"]

    # host-side prep (tiny): padded image, banded weights, broadcast bias
    x_pad = np.zeros((H + HALO, W_PAD), dtype=np.float32)
    x_pad[:H, :W] = X

    wb = np.zeros((128, KW, M_TILE), dtype=np.float32)
    m_idx = np.arange(M_TILE)
    for b in range(KW):
        for a in range(KH):
            wb[m_idx + a, b, m_idx] = weight[a, b]

    bias_bc = np.full((128, 1), bias[0], dtype=np.float32)

    in_maps = [
        {
            "x": x_pad[c * OUT_ROWS : c * OUT_ROWS + IN_ROWS],
            "wb": wb,
            "bias": bias_bc,
        }
        for c in range(N_CORES)
    ]

    res = run_bass_kernel_spmd(nc, in_maps, core_ids=list(range(N_CORES)))
    _CACHE["last_results"] = res  # exec_time_ns when BASS_TRACE=1
    full = np.concatenate([r["out"] for r in res.results], axis=0)
    return np.ascontiguousarray(full[: H - KH + 1, : W - KW + 1])
